# revision 29
# baseline (speedup 1.0000x reference)
"""Sparse multi-head self-attention on 8 trn2 NeuronCores.

Problem: B=4, S=2048, D=768, H=12 heads of 64; only the <=512 keys selected by
`uniform_set` (and not padding-masked) participate in attention.

Sharding: core = 2*b + hg  (b = batch 0..3, hg = head-group 0..1, 6 heads each,
Megatron-style column-sharded Wq/Wk/Wv + row-sharded Wo).  Each core computes a
partial output [S, D] for its batch from its 6 heads; host sums the two
head-group partials per batch (bf16 partials, summed in fp32).

Device algorithm (per core), all layouts transposed so no on-chip transposes;
matmul operands are bf16 (fp32 PSUM accumulation):
  Kt[dout, k]  = WkT . KselT                 (Ksel = gathered selected keys)
  Qt[dout, s]  = WqT^T(chunks) . XT          (XT = query[b].T, host)
  V  [k, dout] = VselT^T . WvT  (+ keep-flag column -> softmax denominator;
                                 flag=0 kills padded keys: their exp(0)=1
                                 never enters numerator (V row = 0) nor den)
  scoresT[k, s] per head, head PAIRS row-packed on the PE (K=64 each, rows
  0-63 / 64-127 concurrent via tile_position) into 4 adjacent PSUM banks
  expT = exp(scoresT)       one ACTIVATE per 4 banks (N=2048, no bias --
                            |scores| ~ O(1), no max subtraction needed)
  cp'T[hd+1, s] = [V|flag]^T . expT   (row 64 = masked sum of exp = den)
  recip = 1/den on DVE; broadcast via two K=1 row+col-packed matmuls
  ctxT = cp'T[0:64] * recip_bcast    (DVE, bf16 out)
  out partial[s_chunk, dout] = ctxT^T . WoT   (bf16 to DRAM)
Biases: bq assumed 0 (reference generates zeros).  bk affects scores only via
per-query constants (softmax invariant).  bv and bo are applied exactly on the
host: out += bo + Wo @ bv (softmax weights sum to 1).
"""

import numpy as np

B, S, D, H, HD = 4, 2048, 768, 12, 64
HG = 2            # head groups (tensor parallel)
HPG = H // HG     # 6 heads per group
DG = HPG * HD     # 384 projection dims per group
NK = 512          # padded count of selected keys
P = 128
KC = D // P       # 6 contraction chunks over model dim
MC = DG // P      # 3 chunks of per-group projection dim = head pairs
SC = NK // P      # 4 selected-key chunks
SQT = 512         # query-tile (moving free dim)
NSQT = S // SQT   # 4
NSTEP = NSQT * MC # 12 (tile, pair) steps

_CACHE = {}
DEBUG = False


def _build_bass():
    import concourse.mybir as mybir
    import concourse.tile as tile
    from concourse import bacc

    f32 = mybir.dt.float32
    bf16 = mybir.dt.bfloat16
    EXP = mybir.ActivationFunctionType.Exp

    nc = bacc.Bacc("TRN2", name="sparse_mha")

    xt_d = nc.dram_tensor("xt", [D, S], bf16, kind="ExternalInput")
    kselt_d = nc.dram_tensor("kselt", [D, NK], bf16, kind="ExternalInput")
    vselt_d = nc.dram_tensor("vselt", [D, NK], bf16, kind="ExternalInput")
    wqt_d = nc.dram_tensor("wqt", [D, DG], bf16, kind="ExternalInput")
    wkt_d = nc.dram_tensor("wkt", [D, DG], bf16, kind="ExternalInput")
    wvt_d = nc.dram_tensor("wvt", [D, DG], bf16, kind="ExternalInput")
    wot_d = nc.dram_tensor("wot", [DG, D], bf16, kind="ExternalInput")
    kf_d = nc.dram_tensor("kflag", [NK], bf16, kind="ExternalInput")
    out_d = nc.dram_tensor("out", [S, D], bf16, kind="ExternalOutput")
    if DEBUG:
        dbg_ets = nc.dram_tensor("dbg_ets", [P, 2, SC, SQT], bf16, kind="ExternalOutput")
        dbg_cp = nc.dram_tensor("dbg_cp", [P, SQT], f32, kind="ExternalOutput")
        dbg_den = nc.dram_tensor("dbg_den", [33, SQT], f32, kind="ExternalOutput")
        dbg_rbs = nc.dram_tensor("dbg_rbs", [P, SQT], f32, kind="ExternalOutput")
        dbg_ctxt = nc.dram_tensor("dbg_ctxt", [P, MC, SQT], bf16, kind="ExternalOutput")
        dbg_qt = nc.dram_tensor("dbg_qt", [P, MC, S], bf16, kind="ExternalOutput")
        dbg_rsf = nc.dram_tensor("dbg_rsf", [33, SQT], f32, kind="ExternalOutput")
        dbg_x1 = nc.dram_tensor("dbg_x1", [HD, SQT], f32, kind="ExternalOutput")
        dbg_x2 = nc.dram_tensor("dbg_x2", [HD, SQT], f32, kind="ExternalOutput")
        dbg_ktp = nc.dram_tensor("dbg_ktp", [P, MC, NK], bf16, kind="ExternalOutput")
        dbg_vb = nc.dram_tensor("dbg_vb", [P, SC, HPG, HD], bf16, kind="ExternalOutput")

    with tile.TileContext(nc) as tc:
        with (
            tc.tile_pool(name="persist", bufs=1) as persist,
            tc.tile_pool(name="inputs", bufs=1) as inputs,
            tc.tile_pool(name="ets", bufs=3) as etsp,
            tc.tile_pool(name="ctxt", bufs=3) as ctxp,
            tc.tile_pool(name="small", bufs=3) as small,
            tc.tile_pool(name="otp", bufs=4) as otp,
            tc.tile_pool(name="ps_sc", bufs=1, space="PSUM") as ps_sc,
            tc.tile_pool(name="ps_cp", bufs=1, space="PSUM") as ps_cp,
            tc.tile_pool(name="ps_den", bufs=1, space="PSUM") as ps_den,
            tc.tile_pool(name="ps_rb", bufs=1, space="PSUM") as ps_rb,
            tc.tile_pool(name="ps_mi", bufs=1, space="PSUM") as ps_mi,
        ):
            # ---- ACT table warm-up: trigger the exp table load at t=0 ----
            warm = persist.tile([1, 8], f32, tag="warm")
            nc.vector.memset(warm, 0.0)
            warm2 = persist.tile([1, 8], f32, tag="warm2")
            nc.scalar.activation(out=warm2, in_=warm, func=EXP)

            # ---- input loads: K-projection operands first ----
            wkt = inputs.tile([P, KC, DG], bf16, tag="wkt")
            nc.scalar.dma_start(wkt, wkt_d.rearrange("(o p) m -> p o m", p=P))
            kselt = inputs.tile([P, KC, NK], bf16, tag="kselt")
            nc.sync.dma_start(kselt, kselt_d.rearrange("(o p) m -> p o m", p=P))
            wqt = inputs.tile([P, KC, DG], bf16, tag="wqt")
            nc.scalar.dma_start(wqt, wqt_d.rearrange("(o p) m -> p o m", p=P))
            xts = []
            for t in range(NSQT):
                xt = inputs.tile([P, KC, SQT], bf16, tag="xt", name=f"xt{t}")
                nc.sync.dma_start(
                    xt,
                    xt_d[:, t * SQT : (t + 1) * SQT].rearrange(
                        "(o p) m -> p o m", p=P
                    ),
                )
                xts.append(xt)
            wvt = inputs.tile([P, KC, DG], bf16, tag="wvt")
            nc.gpsimd.dma_start(wvt, wvt_d.rearrange("(o p) m -> p o m", p=P))
            vselt = inputs.tile([P, KC, NK], bf16, tag="vselt")
            nc.gpsimd.dma_start(vselt, vselt_d.rearrange("(o p) m -> p o m", p=P))
            wot = persist.tile([P, MC, D], bf16, tag="wot")
            nc.gpsimd.dma_start(wot, wot_d.rearrange("(o p) m -> p o m", p=P))
            kflag = persist.tile([P, SC], bf16, tag="kflag")
            nc.scalar.dma_start(kflag, kf_d.rearrange("(c p) -> p c", p=P))

            # ---- persistent SBUF state ----
            qt = persist.tile([P, MC, S], bf16, tag="qt")
            ktp = persist.tile([P, MC, NK], bf16, tag="ktp")
            vb = persist.tile([P, SC, HPG, HD], bf16, tag="vb")
            # lhsT of the K=1 recip-broadcast matmuls (rows 0 and 32 used)
            ones_lh = persist.tile([33, HD], bf16, tag="ones_lh")
            nc.vector.memset(ones_lh, 1.0)

            # ---- K projection: Kt [P(dout), MC, NK] ----
            def kproj():
                for m in range(MC):
                    ps = ps_cp.tile([P, SQT], f32, tag="cp", name=f"kp{m}")
                    for i in range(KC):
                        nc.tensor.matmul(
                            ps,
                            lhsT=wkt[:, i, m * P : (m + 1) * P],
                            rhs=kselt[:, i, :],
                            start=(i == 0),
                            stop=(i == KC - 1),
                        )
                    nc.scalar.copy(ktp[:, m, :], ps)

            # ---- Q projection of one query tile ----
            def qproj(t):
                for m in range(MC):
                    ps = ps_cp.tile([P, SQT], f32, tag="cp", name=f"qp{t}_{m}")
                    for i in range(KC):
                        nc.tensor.matmul(
                            ps,
                            lhsT=wqt[:, i, m * P : (m + 1) * P],
                            rhs=xts[t][:, i, :],
                            start=(i == 0),
                            stop=(i == KC - 1),
                        )
                    nc.scalar.copy(qt[:, m, t * SQT : (t + 1) * SQT], ps)

            # ---- V projection into vb ----
            def vproj(cs):
                for c in cs:
                    ps = ps_mi.tile([P, SQT], f32, tag="mi", name=f"vp{c}")
                    for i in range(KC):
                        nc.tensor.matmul(
                            ps[:, :DG],
                            lhsT=vselt[:, i, c * P : (c + 1) * P],
                            rhs=wvt[:, i, :],
                            start=(i == 0),
                            stop=(i == KC - 1),
                        )
                    nc.vector.tensor_copy(
                        vb[:, c, :, :],
                        ps[:, :DG].rearrange("p (h d) -> p h d", h=HPG),
                    )

            # ---- scores round r of step (t, p): chunks 2r, 2r+1, both heads
            # row-packed (K=64 at rows 0-63 / 64-127), one exp per 4 banks ----
            def s_round(t, p, r, ets):
                sps = ps_sc.tile([P, 2, 2, SQT], f32, tag="sc", name=f"s{t}_{p}_{r}")
                for hi in range(2):
                    lo = HD * hi
                    for ci in range(2):
                        c = 2 * r + ci
                        nc.tensor.matmul(
                            sps[:, hi, ci, :],
                            lhsT=ktp[lo : lo + HD, p, c * P : (c + 1) * P],
                            rhs=qt[lo : lo + HD, p, t * SQT : (t + 1) * SQT],
                            start=True,
                            stop=True,
                            tile_position=(lo, 0),
                        )
                nc.scalar.activation(
                    out=ets[:, :, 2 * r : 2 * r + 2, :], in_=sps, func=EXP
                )

            # ---- ctx matmuls of step (t, p): col-packed head pair (M=64 at
            # output partitions 0-63 / 64-127 of ONE bank) plus col-packed
            # M=1 denominator matmuls (rows 0 / 32 of a second bank) ----
            def ctx_mm(t, p, ets):
                if DEBUG and t == 0 and p == 0:
                    nc.sync.dma_start(dbg_ets[:, :, :, :], ets)
                cp = ps_cp.tile([P, SQT], f32, tag="cp", name=f"c{t}_{p}")
                den = ps_den.tile([33, SQT], f32, tag="den", name=f"d{t}_{p}")
                for c in range(SC):
                    for hi in range(2):
                        nc.tensor.matmul(
                            cp[HD * hi : HD * (hi + 1), :],
                            lhsT=vb[:, c, 2 * p + hi, :],
                            rhs=ets[:, hi, c, :],
                            start=(c == 0),
                            stop=(c == SC - 1),
                            tile_position=(0, HD * hi),
                            skip_group_check=True,
                        )
                for c in range(SC):
                    for hi in range(2):
                        nc.tensor.matmul(
                            den[32 * hi : 32 * hi + 1, :],
                            lhsT=kflag[:, c : c + 1],
                            rhs=ets[:, hi, c, :],
                            start=(c == 0),
                            stop=(c == SC - 1),
                            tile_position=(0, 32 * hi),
                            skip_group_check=True,
                        )
                return cp, den

            # ---- normalize step (t, p): batched recip, 2 gpsimd partition
            # broadcasts (SBUF->SBUF), one pair-wide multiply ----
            def norm(t, p, cpden, ctxt):
                cp, den = cpden
                rsf = small.tile([33, SQT], f32, tag="rsf", name=f"rf{t}_{p}")
                nc.vector.reciprocal(rsf, den)
                rsb = small.tile([33, SQT], bf16, tag="rsb", name=f"rb{t}_{p}")
                nc.gpsimd.tensor_copy(rsb, rsf)
                # broadcast both recip rows via col-packed K=1 matmuls
                # (gpsimd partition_broadcast mishandles partition offsets)
                rb = ps_rb.tile([P, SQT], f32, tag="rb", name=f"rbp{t}_{p}")
                for hi in range(2):
                    nc.tensor.matmul(
                        rb[HD * hi : HD * (hi + 1), :],
                        lhsT=ones_lh[32 * hi : 32 * hi + 1, :],
                        rhs=rsb[32 * hi : 32 * hi + 1, :],
                        start=True,
                        stop=True,
                        tile_position=(32 * hi, HD * hi),
                    )
                rbs = small.tile([P, SQT], f32, tag="rbs", name=f"rbs{t}_{p}")
                nc.vector.tensor_copy(rbs, rb)
                nc.vector.tensor_mul(ctxt[:, p, :], cp, rbs)
                if DEBUG and t == 0 and p == 0:
                    cps_s = persist.tile([P, SQT], f32, tag="dbgcp")
                    nc.vector.tensor_copy(cps_s, cp)
                    nc.sync.dma_start(dbg_cp[:, :], cps_s)
                    den_s = persist.tile([33, SQT], f32, tag="dbgden")
                    nc.vector.tensor_copy(den_s, den)
                    nc.sync.dma_start(dbg_den[:, :], den_s)
                    nc.sync.dma_start(dbg_rbs[:, :], rbs)
                    nc.sync.dma_start(dbg_rsf[:, :], rsf)
                if DEBUG and t == 0 and p == MC - 1:
                    nc.sync.dma_start(dbg_ctxt[:, :, :], ctxt)

            # ---- out-projection of one 128-query stripe of tile t ----
            def oproj_stripe(t, ctxt, mq):
                sq0 = t * SQT + mq * P
                ot = otp.tile([P, D], bf16, tag="ot", name=f"ot{t}_{mq}")
                for n in range(2):
                    ps = ps_mi.tile([P, SQT], f32, tag="mi", name=f"o{t}_{mq}_{n}")
                    for j in range(MC):
                        nc.tensor.matmul(
                            ps[:, :DG],
                            lhsT=ctxt[:, j, mq * P : (mq + 1) * P],
                            rhs=wot[:, j, n * DG : (n + 1) * DG],
                            start=(j == 0),
                            stop=(j == MC - 1),
                        )
                    nc.vector.tensor_copy(ot[:, n * DG : (n + 1) * DG], ps[:, :DG])
                nc.sync.dma_start(out_d[sq0 : sq0 + P, :], ot)

            # ---- schedule ----
            # Steps k = 0..11 map to (t, p) = (k//3, k%3).  Steady-state
            # emission per step: [S r0 | C(prev) | S r1 | R(prev) | filler |
            # stripes] so the in-order PE queue never head-blocks on the exp
            # (r1 of step k waits for exp r0 of step k to release the shared
            # 4-bank scores PSUM tile; C(prev)'s ~1.7us of independent PE
            # work sits between).  Fillers hold the remaining projections;
            # out-proj stripes of tile t start 2 steps after its last norm.
            kproj()
            qproj(0)

            fillers = {
                1: lambda: qproj(1),
                2: lambda: qproj(2),
                4: lambda: qproj(3),
            }
            ostripes = {
                4: (0, (0, 1)),
                5: (0, (2, 3)),
                7: (1, (0, 1)),
                8: (1, (2, 3)),
                10: (2, (0, 1)),
                11: (2, (2, 3)),
            }

            ctxts = {}
            prev = None
            for k in range(NSTEP):
                t, p = divmod(k, MC)
                if p == 0:
                    ctxts[t] = ctxp.tile(
                        [P, MC, SQT], bf16, tag="ctxt", name=f"ctxt{t}"
                    )
                ets = etsp.tile([P, 2, SC, SQT], bf16, tag="ets", name=f"e{t}_{p}")
                s_round(t, p, 0, ets)
                if prev is None:
                    vproj(range(SC))  # fills the exp-r0 shadow at k=0
                else:
                    pt, pp, pets = prev
                    pcps = ctx_mm(pt, pp, pets)
                s_round(t, p, 1, ets)
                if prev is not None:
                    norm(pt, pp, pcps, ctxts[pt])
                if k in fillers:
                    fillers[k]()
                if k in ostripes:
                    ot_t, mqs = ostripes[k]
                    for mq in mqs:
                        oproj_stripe(ot_t, ctxts[ot_t], mq)
                prev = (t, p, ets)
            # tail
            pt, pp, pets = prev
            pcps = ctx_mm(pt, pp, pets)
            norm(pt, pp, pcps, ctxts[pt])
            for mq in range(NSQT):
                oproj_stripe(3, ctxts[3], mq)
            if DEBUG:
                nc.sync.dma_start(dbg_qt[:, :, :], qt)
                nc.sync.dma_start(dbg_ktp[:, :, :], ktp)
                nc.sync.dma_start(dbg_vb[:, :, :, :], vb)

    nc.compile()
    return nc


def _get_nc():
    if "nc" not in _CACHE:
        _CACHE["nc"] = _build_bass()
    return _CACHE["nc"]


def kernel(query, key, value, mask, uniform_set, Wq, bq, Wk, bk, Wv, bv, Wo, bo):
    import ml_dtypes
    from concourse import bass_utils

    bft = ml_dtypes.bfloat16

    query = np.asarray(query, dtype=np.float32)
    key = np.asarray(key, dtype=np.float32)
    value = np.asarray(value, dtype=np.float32)
    mask = np.asarray(mask, dtype=np.float32)
    us = np.asarray(uniform_set).astype(bool)
    Wq = np.asarray(Wq, dtype=np.float32)
    Wk = np.asarray(Wk, dtype=np.float32)
    Wv = np.asarray(Wv, dtype=np.float32)
    Wo = np.asarray(Wo, dtype=np.float32)
    bq = np.asarray(bq, dtype=np.float32)
    bk = np.asarray(bk, dtype=np.float32)
    bv = np.asarray(bv, dtype=np.float32)
    bo = np.asarray(bo, dtype=np.float32)
    assert np.all(bq == 0.0), "kernel assumes bq == 0 (reference generates zeros)"

    nc = _get_nc()

    scale = 1.0 / float(HD) ** 0.5
    wqt_g = [np.ascontiguousarray((Wq.T[:, g * DG : (g + 1) * DG] * scale)).astype(bft) for g in range(HG)]
    wkt_g = [np.ascontiguousarray(Wk.T[:, g * DG : (g + 1) * DG]).astype(bft) for g in range(HG)]
    wvt_g = [np.ascontiguousarray(Wv.T[:, g * DG : (g + 1) * DG]).astype(bft) for g in range(HG)]
    wot_g = [np.ascontiguousarray(Wo.T[g * DG : (g + 1) * DG, :]).astype(bft) for g in range(HG)]

    in_maps = []
    for b in range(B):
        keep = us & (mask[b, 0, 0] >= 0)
        idx = np.nonzero(keep)[0]
        n = len(idx)
        assert 0 < n <= NK, f"selected key count {n} unsupported"
        kselt = np.zeros((D, NK), bft)
        kselt[:, :n] = key[b][idx].T.astype(bft)
        vselt = np.zeros((D, NK), bft)
        vselt[:, :n] = value[b][idx].T.astype(bft)
        kflag = np.zeros((NK,), bft)
        kflag[:n] = 1.0
        xt = np.ascontiguousarray(query[b].T).astype(bft)
        for g in range(HG):
            in_maps.append(
                {
                    "xt": xt,
                    "kselt": kselt,
                    "vselt": vselt,
                    "wqt": wqt_g[g],
                    "wkt": wkt_g[g],
                    "wvt": wvt_g[g],
                    "wot": wot_g[g],
                    "kflag": kflag,
                }
            )

    res = bass_utils.run_bass_kernel_spmd(nc, in_maps, core_ids=list(range(B * HG)))
    outs = [m["out"] for m in res.results]

    corr = (bo + Wo @ bv).astype(np.float32)
    out = np.empty((B, S, D), np.float32)
    for b in range(B):
        out[b] = outs[HG * b].astype(np.float32) + outs[HG * b + 1].astype(np.float32) + corr
    return out


# revision 31
# speedup vs baseline: 1.5325x; 1.5325x over previous
"""Sparse multi-head self-attention on 8 trn2 NeuronCores.

Problem: B=4, S=2048, D=768, H=12 heads of 64; only the <=512 keys selected by
`uniform_set` (and not padding-masked) participate in attention.

Sharding: core = 2*b + hg  (b = batch 0..3, hg = head-group 0..1, 6 heads each,
Megatron-style column-sharded Wq/Wk/Wv + row-sharded Wo).  Each core computes a
partial output [S, D] for its batch from its 6 heads; host sums the two
head-group partials per batch (bf16 partials, summed in fp32).

Device algorithm (per core), all layouts transposed so no on-chip transposes;
matmul operands are bf16 (fp32 PSUM accumulation):
  Kt[dout, k]  = WkT . KselT                 (Ksel = gathered selected keys)
  Qt[dout, s]  = WqT^T(chunks) . XT          (XT = query[b].T, host)
  V  [k, dout] = VselT^T . WvT  (+ keep-flag column -> softmax denominator;
                                 flag=0 kills padded keys: their exp(0)=1
                                 never enters numerator (V row = 0) nor den)
  scoresT[k, s] per head, head PAIRS row-packed on the PE (K=64 each, rows
  0-63 / 64-127 concurrent via tile_position) into 4 adjacent PSUM banks
  expT = exp(scoresT)       one ACTIVATE per 4 banks (N=2048, no bias --
                            |scores| ~ O(1), no max subtraction needed)
  cp'T[hd+1, s] = [V|flag]^T . expT   (row 64 = masked sum of exp = den)
  recip = 1/den on DVE; broadcast via two K=1 row+col-packed matmuls
  ctxT = cp'T[0:64] * recip_bcast    (DVE, bf16 out)
  out partial[s_chunk, dout] = ctxT^T . WoT   (bf16 to DRAM)
Biases: bq assumed 0 (reference generates zeros).  bk affects scores only via
per-query constants (softmax invariant).  bv and bo are applied exactly on the
host: out += bo + Wo @ bv (softmax weights sum to 1).
"""

import numpy as np

B, S, D, H, HD = 4, 2048, 768, 12, 64
HG = 2            # head groups (tensor parallel)
HPG = H // HG     # 6 heads per group
DG = HPG * HD     # 384 projection dims per group
NK = 512          # padded count of selected keys
P = 128
KC = D // P       # 6 contraction chunks over model dim
MC = DG // P      # 3 chunks of per-group projection dim = head pairs
SC = NK // P      # 4 selected-key chunks
SQT = 512         # query-tile (moving free dim)
NSQT = S // SQT   # 4
NSTEP = NSQT * MC # 12 (tile, pair) steps

_CACHE = {}
DEBUG = False


def _build_bass():
    import concourse.mybir as mybir
    import concourse.tile as tile
    from concourse import bacc

    f32 = mybir.dt.float32
    bf16 = mybir.dt.bfloat16
    EXP = mybir.ActivationFunctionType.Exp

    nc = bacc.Bacc("TRN2", name="sparse_mha")

    xt_d = nc.dram_tensor("xt", [D, S], bf16, kind="ExternalInput")
    kselt_d = nc.dram_tensor("kselt", [D, NK], bf16, kind="ExternalInput")
    vselt_d = nc.dram_tensor("vselt", [D, NK], bf16, kind="ExternalInput")
    wqt_d = nc.dram_tensor("wqt", [D, DG], bf16, kind="ExternalInput")
    wkt_d = nc.dram_tensor("wkt", [D, DG], bf16, kind="ExternalInput")
    wvt_d = nc.dram_tensor("wvt", [D, DG], bf16, kind="ExternalInput")
    wot_d = nc.dram_tensor("wot", [DG, D], bf16, kind="ExternalInput")
    kf_d = nc.dram_tensor("kflag", [NK], bf16, kind="ExternalInput")
    out_d = nc.dram_tensor("out", [S, D], bf16, kind="ExternalOutput")
    if DEBUG:
        dbg_ets = nc.dram_tensor("dbg_ets", [P, 2, SC, SQT], bf16, kind="ExternalOutput")
        dbg_cp = nc.dram_tensor("dbg_cp", [P, SQT], f32, kind="ExternalOutput")
        dbg_den = nc.dram_tensor("dbg_den", [33, SQT], f32, kind="ExternalOutput")
        dbg_rbs = nc.dram_tensor("dbg_rbs", [P, SQT], f32, kind="ExternalOutput")
        dbg_ctxt = nc.dram_tensor("dbg_ctxt", [P, MC, SQT], bf16, kind="ExternalOutput")
        dbg_qt = nc.dram_tensor("dbg_qt", [P, MC, S], bf16, kind="ExternalOutput")
        dbg_rsf = nc.dram_tensor("dbg_rsf", [33, SQT], f32, kind="ExternalOutput")
        dbg_x1 = nc.dram_tensor("dbg_x1", [HD, SQT], f32, kind="ExternalOutput")
        dbg_x2 = nc.dram_tensor("dbg_x2", [HD, SQT], f32, kind="ExternalOutput")
        dbg_ktp = nc.dram_tensor("dbg_ktp", [P, MC, NK], bf16, kind="ExternalOutput")
        dbg_vb = nc.dram_tensor("dbg_vb", [P, SC, HPG, HD], bf16, kind="ExternalOutput")

    with tile.TileContext(nc) as tc:
        with (
            tc.tile_pool(name="persist", bufs=1) as persist,
            tc.tile_pool(name="inputs", bufs=1) as inputs,
            tc.tile_pool(name="ets", bufs=3) as etsp,
            tc.tile_pool(name="ctxt", bufs=3) as ctxp,
            tc.tile_pool(name="small", bufs=3) as small,
            tc.tile_pool(name="otp", bufs=4) as otp,
            tc.tile_pool(name="ps_sc", bufs=1, space="PSUM") as ps_sc,
            tc.tile_pool(name="ps_cp", bufs=1, space="PSUM") as ps_cp,
            tc.tile_pool(name="ps_den", bufs=1, space="PSUM") as ps_den,
            tc.tile_pool(name="ps_rb", bufs=1, space="PSUM") as ps_rb,
            tc.tile_pool(name="ps_mi", bufs=1, space="PSUM") as ps_mi,
        ):
            # ---- ACT table warm-up: trigger the exp table load at t=0 ----
            warm = persist.tile([1, 8], f32, tag="warm")
            nc.vector.memset(warm, 0.0)
            warm2 = persist.tile([1, 8], f32, tag="warm2")
            nc.scalar.activation(out=warm2, in_=warm, func=EXP)

            # ---- input loads: K-projection operands first ----
            wkt = inputs.tile([P, KC, DG], bf16, tag="wkt")
            nc.scalar.dma_start(wkt, wkt_d.rearrange("(o p) m -> p o m", p=P))
            kselt = inputs.tile([P, KC, NK], bf16, tag="kselt")
            nc.sync.dma_start(kselt, kselt_d.rearrange("(o p) m -> p o m", p=P))
            wqt = inputs.tile([P, KC, DG], bf16, tag="wqt")
            nc.scalar.dma_start(wqt, wqt_d.rearrange("(o p) m -> p o m", p=P))
            xts = []
            for t in range(NSQT):
                xt = inputs.tile([P, KC, SQT], bf16, tag="xt", name=f"xt{t}")
                nc.sync.dma_start(
                    xt,
                    xt_d[:, t * SQT : (t + 1) * SQT].rearrange(
                        "(o p) m -> p o m", p=P
                    ),
                )
                xts.append(xt)
            wvt = inputs.tile([P, KC, DG], bf16, tag="wvt")
            nc.gpsimd.dma_start(wvt, wvt_d.rearrange("(o p) m -> p o m", p=P))
            vselt = inputs.tile([P, KC, NK], bf16, tag="vselt")
            nc.gpsimd.dma_start(vselt, vselt_d.rearrange("(o p) m -> p o m", p=P))
            wot = persist.tile([P, MC, D], bf16, tag="wot")
            nc.gpsimd.dma_start(wot, wot_d.rearrange("(o p) m -> p o m", p=P))
            kflag = persist.tile([P, SC], bf16, tag="kflag")
            nc.scalar.dma_start(kflag, kf_d.rearrange("(c p) -> p c", p=P))

            # ---- persistent SBUF state ----
            qt = persist.tile([P, MC, S], bf16, tag="qt")
            ktp = persist.tile([P, MC, NK], bf16, tag="ktp")
            vb = persist.tile([P, SC, HPG, HD], bf16, tag="vb")
            # lhsT of the K=1 recip-broadcast matmuls (rows 0 and 32 used)
            ones_lh = persist.tile([33, HD], bf16, tag="ones_lh")
            nc.vector.memset(ones_lh, 1.0)

            # ---- K projection: Kt [P(dout), MC, NK] ----
            def kproj():
                for m in range(MC):
                    ps = ps_cp.tile([P, SQT], f32, tag="cp", name=f"kp{m}")
                    for i in range(KC):
                        nc.tensor.matmul(
                            ps,
                            lhsT=wkt[:, i, m * P : (m + 1) * P],
                            rhs=kselt[:, i, :],
                            start=(i == 0),
                            stop=(i == KC - 1),
                        )
                    nc.scalar.copy(ktp[:, m, :], ps)

            # ---- Q projection of one query tile ----
            def qproj(t):
                for m in range(MC):
                    ps = ps_cp.tile([P, SQT], f32, tag="cp", name=f"qp{t}_{m}")
                    for i in range(KC):
                        nc.tensor.matmul(
                            ps,
                            lhsT=wqt[:, i, m * P : (m + 1) * P],
                            rhs=xts[t][:, i, :],
                            start=(i == 0),
                            stop=(i == KC - 1),
                        )
                    nc.scalar.copy(qt[:, m, t * SQT : (t + 1) * SQT], ps)

            # ---- V projection into vb ----
            def vproj(cs):
                for c in cs:
                    ps = ps_mi.tile([P, SQT], f32, tag="mi", name=f"vp{c}")
                    for i in range(KC):
                        nc.tensor.matmul(
                            ps[:, :DG],
                            lhsT=vselt[:, i, c * P : (c + 1) * P],
                            rhs=wvt[:, i, :],
                            start=(i == 0),
                            stop=(i == KC - 1),
                        )
                    nc.vector.tensor_copy(
                        vb[:, c, :, :],
                        ps[:, :DG].rearrange("p (h d) -> p h d", h=HPG),
                    )

            # ---- scores round r of step (t, p): chunks 2r, 2r+1, both heads
            # row-packed (K=64 at rows 0-63 / 64-127), one exp per 4 banks ----
            def s_round(t, p, r, ets):
                sps = ps_sc.tile([P, 2, 2, SQT], f32, tag="sc", name=f"s{t}_{p}_{r}")
                for hi in range(2):
                    lo = HD * hi
                    for ci in range(2):
                        c = 2 * r + ci
                        nc.tensor.matmul(
                            sps[:, hi, ci, :],
                            lhsT=ktp[lo : lo + HD, p, c * P : (c + 1) * P],
                            rhs=qt[lo : lo + HD, p, t * SQT : (t + 1) * SQT],
                            start=True,
                            stop=True,
                            tile_position=(lo, 0),
                        )
                nc.scalar.activation(
                    out=ets[:, :, 2 * r : 2 * r + 2, :], in_=sps, func=EXP
                )

            # ---- ctx matmuls of step (t, p): col-packed head pair (M=64 at
            # output partitions 0-63 / 64-127 of ONE bank) plus col-packed
            # M=1 denominator matmuls (rows 0 / 32 of a second bank) ----
            def ctx_mm(t, p, ets):
                if DEBUG and t == 0 and p == 0:
                    nc.sync.dma_start(dbg_ets[:, :, :, :], ets)
                cp = ps_cp.tile([P, SQT], f32, tag="cp", name=f"c{t}_{p}")
                den = ps_den.tile([33, SQT], f32, tag="den", name=f"d{t}_{p}")
                for c in range(SC):
                    for hi in range(2):
                        nc.tensor.matmul(
                            cp[HD * hi : HD * (hi + 1), :],
                            lhsT=vb[:, c, 2 * p + hi, :],
                            rhs=ets[:, hi, c, :],
                            start=(c == 0),
                            stop=(c == SC - 1),
                            tile_position=(0, HD * hi),
                            skip_group_check=True,
                        )
                for c in range(SC):
                    for hi in range(2):
                        nc.tensor.matmul(
                            den[32 * hi : 32 * hi + 1, :],
                            lhsT=kflag[:, c : c + 1],
                            rhs=ets[:, hi, c, :],
                            start=(c == 0),
                            stop=(c == SC - 1),
                            tile_position=(0, 32 * hi),
                            skip_group_check=True,
                        )
                return cp, den

            # ---- normalize step (t, p): batched recip, 2 gpsimd partition
            # broadcasts (SBUF->SBUF), one pair-wide multiply ----
            def norm(t, p, cpden, ctxt):
                cp, den = cpden
                # broadcast both den rows via col-packed K=1 matmuls
                # (gpsimd partition_broadcast mishandles partition offsets),
                # then approx-reciprocal the broadcast (5x faster than
                # nc.vector.reciprocal; ~18 bits, plenty for the softmax den)
                rsb = small.tile([33, SQT], bf16, tag="rsb", name=f"rb{t}_{p}")
                nc.vector.tensor_copy(rsb, den)
                rb = ps_rb.tile([P, SQT], f32, tag="rb", name=f"rbp{t}_{p}")
                for hi in range(2):
                    nc.tensor.matmul(
                        rb[HD * hi : HD * (hi + 1), :],
                        lhsT=ones_lh[32 * hi : 32 * hi + 1, :],
                        rhs=rsb[32 * hi : 32 * hi + 1, :],
                        start=True,
                        stop=True,
                        tile_position=(32 * hi, HD * hi),
                    )
                rbs = small.tile([P, SQT], f32, tag="rbs", name=f"rbs{t}_{p}")
                nc.vector.reciprocal_approx_fast(rbs, rb)
                nc.vector.tensor_mul(ctxt[:, p, :], cp, rbs)
                if DEBUG and t == 0 and p == 0:
                    cps_s = persist.tile([P, SQT], f32, tag="dbgcp")
                    nc.vector.tensor_copy(cps_s, cp)
                    nc.sync.dma_start(dbg_cp[:, :], cps_s)
                    den_s = persist.tile([33, SQT], f32, tag="dbgden")
                    nc.vector.tensor_copy(den_s, den)
                    nc.sync.dma_start(dbg_den[:, :], den_s)
                    nc.sync.dma_start(dbg_rbs[:, :], rbs)
                if DEBUG and t == 0 and p == MC - 1:
                    nc.sync.dma_start(dbg_ctxt[:, :, :], ctxt)

            # ---- out-projection of one 128-query stripe of tile t ----
            def oproj_stripe(t, ctxt, mq):
                sq0 = t * SQT + mq * P
                ot = otp.tile([P, D], bf16, tag="ot", name=f"ot{t}_{mq}")
                for n in range(2):
                    ps = ps_mi.tile([P, SQT], f32, tag="mi", name=f"o{t}_{mq}_{n}")
                    for j in range(MC):
                        nc.tensor.matmul(
                            ps[:, :DG],
                            lhsT=ctxt[:, j, mq * P : (mq + 1) * P],
                            rhs=wot[:, j, n * DG : (n + 1) * DG],
                            start=(j == 0),
                            stop=(j == MC - 1),
                        )
                    nc.vector.tensor_copy(ot[:, n * DG : (n + 1) * DG], ps[:, :DG])
                nc.sync.dma_start(out_d[sq0 : sq0 + P, :], ot)

            # ---- schedule ----
            # Steps k = 0..11 map to (t, p) = (k//3, k%3).  Steady-state
            # emission per step: [S r0 | C(prev) | S r1 | R(prev) | filler |
            # stripes] so the in-order PE queue never head-blocks on the exp
            # (r1 of step k waits for exp r0 of step k to release the shared
            # 4-bank scores PSUM tile; C(prev)'s ~1.7us of independent PE
            # work sits between).  Fillers hold the remaining projections;
            # out-proj stripes of tile t start 2 steps after its last norm.
            kproj()
            qproj(0)

            fillers = {
                1: lambda: qproj(1),
                2: lambda: qproj(2),
                4: lambda: qproj(3),
            }
            ostripes = {
                4: (0, (0, 1)),
                5: (0, (2, 3)),
                7: (1, (0, 1)),
                8: (1, (2, 3)),
                10: (2, (0, 1)),
                11: (2, (2, 3)),
            }

            ctxts = {}
            prev = None
            for k in range(NSTEP):
                t, p = divmod(k, MC)
                if p == 0:
                    ctxts[t] = ctxp.tile(
                        [P, MC, SQT], bf16, tag="ctxt", name=f"ctxt{t}"
                    )
                ets = etsp.tile([P, 2, SC, SQT], bf16, tag="ets", name=f"e{t}_{p}")
                s_round(t, p, 0, ets)
                if prev is None:
                    vproj(range(SC))  # fills the exp-r0 shadow at k=0
                else:
                    pt, pp, pets = prev
                    pcps = ctx_mm(pt, pp, pets)
                s_round(t, p, 1, ets)
                if prev is not None:
                    norm(pt, pp, pcps, ctxts[pt])
                if k in fillers:
                    fillers[k]()
                if k in ostripes:
                    ot_t, mqs = ostripes[k]
                    for mq in mqs:
                        oproj_stripe(ot_t, ctxts[ot_t], mq)
                prev = (t, p, ets)
            # tail
            pt, pp, pets = prev
            pcps = ctx_mm(pt, pp, pets)
            norm(pt, pp, pcps, ctxts[pt])
            for mq in range(NSQT):
                oproj_stripe(3, ctxts[3], mq)
            if DEBUG:
                nc.sync.dma_start(dbg_qt[:, :, :], qt)
                nc.sync.dma_start(dbg_ktp[:, :, :], ktp)
                nc.sync.dma_start(dbg_vb[:, :, :, :], vb)

    nc.compile()
    return nc


def _get_nc():
    if "nc" not in _CACHE:
        _CACHE["nc"] = _build_bass()
    return _CACHE["nc"]


def kernel(query, key, value, mask, uniform_set, Wq, bq, Wk, bk, Wv, bv, Wo, bo):
    import ml_dtypes
    from concourse import bass_utils

    bft = ml_dtypes.bfloat16

    query = np.asarray(query, dtype=np.float32)
    key = np.asarray(key, dtype=np.float32)
    value = np.asarray(value, dtype=np.float32)
    mask = np.asarray(mask, dtype=np.float32)
    us = np.asarray(uniform_set).astype(bool)
    Wq = np.asarray(Wq, dtype=np.float32)
    Wk = np.asarray(Wk, dtype=np.float32)
    Wv = np.asarray(Wv, dtype=np.float32)
    Wo = np.asarray(Wo, dtype=np.float32)
    bq = np.asarray(bq, dtype=np.float32)
    bk = np.asarray(bk, dtype=np.float32)
    bv = np.asarray(bv, dtype=np.float32)
    bo = np.asarray(bo, dtype=np.float32)
    assert np.all(bq == 0.0), "kernel assumes bq == 0 (reference generates zeros)"

    nc = _get_nc()

    scale = 1.0 / float(HD) ** 0.5
    wqt_g = [np.ascontiguousarray((Wq.T[:, g * DG : (g + 1) * DG] * scale)).astype(bft) for g in range(HG)]
    wkt_g = [np.ascontiguousarray(Wk.T[:, g * DG : (g + 1) * DG]).astype(bft) for g in range(HG)]
    wvt_g = [np.ascontiguousarray(Wv.T[:, g * DG : (g + 1) * DG]).astype(bft) for g in range(HG)]
    wot_g = [np.ascontiguousarray(Wo.T[g * DG : (g + 1) * DG, :]).astype(bft) for g in range(HG)]

    in_maps = []
    for b in range(B):
        keep = us & (mask[b, 0, 0] >= 0)
        idx = np.nonzero(keep)[0]
        n = len(idx)
        assert 0 < n <= NK, f"selected key count {n} unsupported"
        kselt = np.zeros((D, NK), bft)
        kselt[:, :n] = key[b][idx].T.astype(bft)
        vselt = np.zeros((D, NK), bft)
        vselt[:, :n] = value[b][idx].T.astype(bft)
        kflag = np.zeros((NK,), bft)
        kflag[:n] = 1.0
        xt = np.ascontiguousarray(query[b].T).astype(bft)
        for g in range(HG):
            in_maps.append(
                {
                    "xt": xt,
                    "kselt": kselt,
                    "vselt": vselt,
                    "wqt": wqt_g[g],
                    "wkt": wkt_g[g],
                    "wvt": wvt_g[g],
                    "wot": wot_g[g],
                    "kflag": kflag,
                }
            )

    res = bass_utils.run_bass_kernel_spmd(nc, in_maps, core_ids=list(range(B * HG)))
    outs = [m["out"] for m in res.results]

    corr = (bo + Wo @ bv).astype(np.float32)
    out = np.empty((B, S, D), np.float32)
    for b in range(B):
        out[b] = outs[HG * b].astype(np.float32) + outs[HG * b + 1].astype(np.float32) + corr
    return out


# revision 33
# speedup vs baseline: 1.6162x; 1.0546x over previous
"""Sparse multi-head self-attention on 8 trn2 NeuronCores.

Problem: B=4, S=2048, D=768, H=12 heads of 64; only the <=512 keys selected by
`uniform_set` (and not padding-masked) participate in attention.

Sharding: core = 2*b + hg  (b = batch 0..3, hg = head-group 0..1, 6 heads each,
Megatron-style column-sharded Wq/Wk/Wv + row-sharded Wo).  Each core computes a
partial output [S, D] for its batch from its 6 heads; host sums the two
head-group partials per batch (bf16 partials, summed in fp32).

Device algorithm (per core), all layouts transposed so no on-chip transposes;
matmul operands are bf16 (fp32 PSUM accumulation):
  Kt[dout, k]  = WkT . KselT                 (Ksel = gathered selected keys)
  Qt[dout, s]  = WqT^T(chunks) . XT          (XT = query[b].T, host)
  V  [k, dout] = VselT^T . WvT  (+ keep-flag column -> softmax denominator;
                                 flag=0 kills padded keys: their exp(0)=1
                                 never enters numerator (V row = 0) nor den)
  scoresT[k, s] per head, head PAIRS row-packed on the PE (K=64 each, rows
  0-63 / 64-127 concurrent via tile_position) into 4 adjacent PSUM banks
  expT = exp(scoresT)       one ACTIVATE per 4 banks (N=2048, no bias --
                            |scores| ~ O(1), no max subtraction needed)
  cp'T[hd+1, s] = [V|flag]^T . expT   (row 64 = masked sum of exp = den)
  recip = 1/den on DVE; broadcast via two K=1 row+col-packed matmuls
  ctxT = cp'T[0:64] * recip_bcast    (DVE, bf16 out)
  out partial[s_chunk, dout] = ctxT^T . WoT   (bf16 to DRAM)
Biases: bq assumed 0 (reference generates zeros).  bk affects scores only via
per-query constants (softmax invariant).  bv and bo are applied exactly on the
host: out += bo + Wo @ bv (softmax weights sum to 1).
"""

import numpy as np

B, S, D, H, HD = 4, 2048, 768, 12, 64
HG = 2            # head groups (tensor parallel)
HPG = H // HG     # 6 heads per group
DG = HPG * HD     # 384 projection dims per group
NK = 512          # padded count of selected keys
P = 128
KC = D // P       # 6 contraction chunks over model dim
MC = DG // P      # 3 chunks of per-group projection dim = head pairs
SC = NK // P      # 4 selected-key chunks
SQT = 512         # query-tile (moving free dim)
NSQT = S // SQT   # 4
NSTEP = NSQT * MC # 12 (tile, pair) steps

_CACHE = {}
DEBUG = False


def _build_bass():
    import concourse.mybir as mybir
    import concourse.tile as tile
    from concourse import bacc

    f32 = mybir.dt.float32
    bf16 = mybir.dt.bfloat16
    EXP = mybir.ActivationFunctionType.Exp

    nc = bacc.Bacc("TRN2", name="sparse_mha")

    xt_d = nc.dram_tensor("xt", [D, S], bf16, kind="ExternalInput")
    kselt_d = nc.dram_tensor("kselt", [D, NK], bf16, kind="ExternalInput")
    vselt_d = nc.dram_tensor("vselt", [D, NK], bf16, kind="ExternalInput")
    wqt_d = nc.dram_tensor("wqt", [D, DG], bf16, kind="ExternalInput")
    wkt_d = nc.dram_tensor("wkt", [D, DG], bf16, kind="ExternalInput")
    wvt_d = nc.dram_tensor("wvt", [D, DG], bf16, kind="ExternalInput")
    wot_d = nc.dram_tensor("wot", [DG, D], bf16, kind="ExternalInput")
    kf_d = nc.dram_tensor("kflag", [NK], bf16, kind="ExternalInput")
    out_d = nc.dram_tensor("out", [S, D], bf16, kind="ExternalOutput")
    if DEBUG:
        dbg_ets = nc.dram_tensor("dbg_ets", [P, 2, SC, SQT], bf16, kind="ExternalOutput")
        dbg_cp = nc.dram_tensor("dbg_cp", [P, SQT], f32, kind="ExternalOutput")
        dbg_den = nc.dram_tensor("dbg_den", [33, SQT], f32, kind="ExternalOutput")
        dbg_rbs = nc.dram_tensor("dbg_rbs", [P, SQT], f32, kind="ExternalOutput")
        dbg_ctxt = nc.dram_tensor("dbg_ctxt", [P, MC, SQT], bf16, kind="ExternalOutput")
        dbg_qt = nc.dram_tensor("dbg_qt", [P, MC, S], bf16, kind="ExternalOutput")
        dbg_rsf = nc.dram_tensor("dbg_rsf", [33, SQT], f32, kind="ExternalOutput")
        dbg_x1 = nc.dram_tensor("dbg_x1", [HD, SQT], f32, kind="ExternalOutput")
        dbg_x2 = nc.dram_tensor("dbg_x2", [HD, SQT], f32, kind="ExternalOutput")
        dbg_ktp = nc.dram_tensor("dbg_ktp", [P, MC, NK], bf16, kind="ExternalOutput")
        dbg_vb = nc.dram_tensor("dbg_vb", [P, SC, HPG, HD], bf16, kind="ExternalOutput")

    with tile.TileContext(nc) as tc:
        with (
            tc.tile_pool(name="persist", bufs=1) as persist,
            tc.tile_pool(name="inputs", bufs=1) as inputs,
            tc.tile_pool(name="ets", bufs=3) as etsp,
            tc.tile_pool(name="ctxt", bufs=3) as ctxp,
            tc.tile_pool(name="small", bufs=3) as small,
            tc.tile_pool(name="otp", bufs=4) as otp,
            tc.tile_pool(name="ps_sc", bufs=1, space="PSUM") as ps_sc,
            tc.tile_pool(name="ps_cp", bufs=1, space="PSUM") as ps_cp,
            tc.tile_pool(name="ps_dr", bufs=1, space="PSUM") as ps_dr,
            tc.tile_pool(name="ps_mi", bufs=2, space="PSUM") as ps_mi,
        ):
            # ---- input loads first: K-projection operands lead, split in
            # half along the contraction so the first matmuls start early ----
            wkt = inputs.tile([P, KC, DG], bf16, tag="wkt")
            wkt_r = wkt_d.rearrange("(o p) m -> p o m", p=P)
            nc.scalar.dma_start(wkt[:, 0:3, :], wkt_r[:, 0:3, :])
            kselt = inputs.tile([P, KC, NK], bf16, tag="kselt")
            kselt_r = kselt_d.rearrange("(o p) m -> p o m", p=P)
            nc.sync.dma_start(kselt[:, 0:3, :], kselt_r[:, 0:3, :])
            nc.scalar.dma_start(wkt[:, 3:6, :], wkt_r[:, 3:6, :])
            nc.sync.dma_start(kselt[:, 3:6, :], kselt_r[:, 3:6, :])
            wqt = inputs.tile([P, KC, DG], bf16, tag="wqt")
            nc.scalar.dma_start(wqt, wqt_d.rearrange("(o p) m -> p o m", p=P))
            xts = []
            xt_engs = [nc.sync, nc.scalar, nc.gpsimd, nc.sync]
            for t in range(NSQT):
                xt = inputs.tile([P, KC, SQT], bf16, tag="xt", name=f"xt{t}")
                xts.append(xt)
            nc.sync.dma_start(
                xts[0], xt_d[:, 0:SQT].rearrange("(o p) m -> p o m", p=P)
            )
            wvt = inputs.tile([P, KC, DG], bf16, tag="wvt")
            nc.gpsimd.dma_start(wvt, wvt_d.rearrange("(o p) m -> p o m", p=P))
            vselt = inputs.tile([P, KC, NK], bf16, tag="vselt")
            nc.gpsimd.dma_start(vselt, vselt_d.rearrange("(o p) m -> p o m", p=P))
            kflag = persist.tile([P, SC], bf16, tag="kflag")
            nc.scalar.dma_start(kflag, kf_d.rearrange("(c p) -> p c", p=P))
            for t in range(1, NSQT):
                xt_engs[t].dma_start(
                    xts[t],
                    xt_d[:, t * SQT : (t + 1) * SQT].rearrange(
                        "(o p) m -> p o m", p=P
                    ),
                )
            wot = persist.tile([P, MC, D], bf16, tag="wot")
            nc.gpsimd.dma_start(wot, wot_d.rearrange("(o p) m -> p o m", p=P))

            # ---- warm-ups: exp table load on ACT, dummy matmuls to flip the
            # HAM clock gate to 8/8 before the real matmuls arrive ----
            warmpe = persist.tile([P, SQT], bf16, tag="warmpe")
            nc.vector.memset(warmpe, 0.0)
            warm2 = persist.tile([1, 8], f32, tag="warm2")
            nc.scalar.activation(out=warm2, in_=warmpe[0:1, 0:8], func=EXP)
            wps = ps_mi.tile([P, SQT], f32, tag="mi", name="warmps")
            for i in range(16):
                nc.tensor.matmul(
                    wps, lhsT=warmpe[:, 0:P], rhs=warmpe, start=True, stop=True
                )

            # ---- persistent SBUF state ----
            qt = persist.tile([P, MC, S], bf16, tag="qt")
            ktp = persist.tile([P, MC, NK], bf16, tag="ktp")
            vb = persist.tile([P, SC, HPG, HD], bf16, tag="vb")
            # lhsT of the K=1 den-broadcast matmuls (rows 0 and 32 used)
            ones_lh = persist.tile([33, HD], bf16, tag="ones_lh")
            nc.vector.memset(ones_lh, 1.0)

            # ---- K projection: Kt [P(dout), MC, NK] ----
            def kproj():
                for m in range(MC):
                    ps = ps_mi.tile([P, SQT], f32, tag="mi", name=f"kp{m}")
                    for i in range(KC):
                        nc.tensor.matmul(
                            ps,
                            lhsT=wkt[:, i, m * P : (m + 1) * P],
                            rhs=kselt[:, i, :],
                            start=(i == 0),
                            stop=(i == KC - 1),
                        )
                    nc.scalar.copy(ktp[:, m, :], ps)

            # ---- Q projection of one query tile ----
            def qproj(t):
                for m in range(MC):
                    ps = ps_mi.tile([P, SQT], f32, tag="mi", name=f"qp{t}_{m}")
                    for i in range(KC):
                        nc.tensor.matmul(
                            ps,
                            lhsT=wqt[:, i, m * P : (m + 1) * P],
                            rhs=xts[t][:, i, :],
                            start=(i == 0),
                            stop=(i == KC - 1),
                        )
                    nc.scalar.copy(qt[:, m, t * SQT : (t + 1) * SQT], ps)

            # ---- V projection into vb ----
            def vproj(cs):
                for c in cs:
                    ps = ps_mi.tile([P, SQT], f32, tag="mi", name=f"vp{c}")
                    for i in range(KC):
                        nc.tensor.matmul(
                            ps[:, :DG],
                            lhsT=vselt[:, i, c * P : (c + 1) * P],
                            rhs=wvt[:, i, :],
                            start=(i == 0),
                            stop=(i == KC - 1),
                        )
                    nc.vector.tensor_copy(
                        vb[:, c, :, :],
                        ps[:, :DG].rearrange("p (h d) -> p h d", h=HPG),
                    )

            # ---- scores round r of step (t, p): chunks 2r, 2r+1, both heads
            # row-packed (K=64 at rows 0-63 / 64-127), one exp per 4 banks ----
            def s_round(t, p, r, ets):
                sps = ps_sc.tile([P, 2, 2, SQT], f32, tag="sc", name=f"s{t}_{p}_{r}")
                for hi in range(2):
                    lo = HD * hi
                    for ci in range(2):
                        c = 2 * r + ci
                        nc.tensor.matmul(
                            sps[:, hi, ci, :],
                            lhsT=ktp[lo : lo + HD, p, c * P : (c + 1) * P],
                            rhs=qt[lo : lo + HD, p, t * SQT : (t + 1) * SQT],
                            start=True,
                            stop=True,
                            tile_position=(lo, 0),
                        )
                nc.scalar.activation(
                    out=ets[:, :, 2 * r : 2 * r + 2, :], in_=sps, func=EXP
                )

            # ---- ctx matmuls of step (t, p): col-packed head pair (M=64 at
            # output partitions 0-63 / 64-127 of ONE bank) plus col-packed
            # M=1 denominator matmuls (rows 0 / 32 of a second bank) ----
            def ctx_mm(t, p, ets):
                if DEBUG and t == 0 and p == 0:
                    nc.sync.dma_start(dbg_ets[:, :, :, :], ets)
                cp = ps_cp.tile([P, SQT], f32, tag="cp", name=f"c{t}_{p}")
                den = ps_dr.tile([P, SQT], f32, tag="dr", name=f"d{t}_{p}")
                for c in range(SC):
                    for hi in range(2):
                        nc.tensor.matmul(
                            cp[HD * hi : HD * (hi + 1), :],
                            lhsT=vb[:, c, 2 * p + hi, :],
                            rhs=ets[:, hi, c, :],
                            start=(c == 0),
                            stop=(c == SC - 1),
                            tile_position=(0, HD * hi),
                            skip_group_check=True,
                        )
                for c in range(SC):
                    for hi in range(2):
                        nc.tensor.matmul(
                            den[32 * hi : 32 * hi + 1, :],
                            lhsT=kflag[:, c : c + 1],
                            rhs=ets[:, hi, c, :],
                            start=(c == 0),
                            stop=(c == SC - 1),
                            tile_position=(0, 32 * hi),
                            skip_group_check=True,
                        )
                return cp, den

            # ---- normalize step (t, p): batched recip, 2 gpsimd partition
            # broadcasts (SBUF->SBUF), one pair-wide multiply ----
            def norm(t, p, cpden, ctxt):
                cp, den = cpden
                # broadcast both den rows via col-packed K=1 matmuls
                # (gpsimd partition_broadcast mishandles partition offsets),
                # then approx-reciprocal the broadcast (5x faster than
                # nc.vector.reciprocal; ~18 bits, plenty for the softmax den)
                rsb = small.tile([33, SQT], bf16, tag="rsb", name=f"rb{t}_{p}")
                nc.vector.tensor_copy(rsb, den[0:33, :])
                rb = ps_dr.tile([P, SQT], f32, tag="dr", name=f"rbp{t}_{p}")
                for hi in range(2):
                    nc.tensor.matmul(
                        rb[HD * hi : HD * (hi + 1), :],
                        lhsT=ones_lh[32 * hi : 32 * hi + 1, :],
                        rhs=rsb[32 * hi : 32 * hi + 1, :],
                        start=True,
                        stop=True,
                        tile_position=(32 * hi, HD * hi),
                    )
                rbs = small.tile([P, SQT], f32, tag="rbs", name=f"rbs{t}_{p}")
                nc.vector.reciprocal_approx_fast(rbs, rb)
                nc.vector.tensor_mul(ctxt[:, p, :], cp, rbs)
                if DEBUG and t == 0 and p == 0:
                    cps_s = persist.tile([P, SQT], f32, tag="dbgcp")
                    nc.vector.tensor_copy(cps_s, cp)
                    nc.sync.dma_start(dbg_cp[:, :], cps_s)
                    den_s = persist.tile([33, SQT], f32, tag="dbgden")
                    nc.vector.tensor_copy(den_s, den[0:33, :])
                    nc.sync.dma_start(dbg_den[:, :], den_s)
                    nc.sync.dma_start(dbg_rbs[:, :], rbs)
                if DEBUG and t == 0 and p == MC - 1:
                    nc.sync.dma_start(dbg_ctxt[:, :, :], ctxt)

            # ---- out-projection of one 128-query stripe of tile t ----
            def oproj_stripe(t, ctxt, mq):
                sq0 = t * SQT + mq * P
                ot = otp.tile([P, D], bf16, tag="ot", name=f"ot{t}_{mq}")
                for n in range(2):
                    ps = ps_mi.tile([P, SQT], f32, tag="mi", name=f"o{t}_{mq}_{n}")
                    for j in range(MC):
                        nc.tensor.matmul(
                            ps[:, :DG],
                            lhsT=ctxt[:, j, mq * P : (mq + 1) * P],
                            rhs=wot[:, j, n * DG : (n + 1) * DG],
                            start=(j == 0),
                            stop=(j == MC - 1),
                        )
                    nc.vector.tensor_copy(ot[:, n * DG : (n + 1) * DG], ps[:, :DG])
                nc.sync.dma_start(out_d[sq0 : sq0 + P, :], ot)

            # ---- schedule ----
            # Steps k = 0..11 map to (t, p) = (k//3, k%3).  Steady-state
            # emission per step: [S r0 | C(prev) | S r1 | R(prev) | filler |
            # stripes] so the in-order PE queue never head-blocks on the exp
            # (r1 of step k waits for exp r0 of step k to release the shared
            # 4-bank scores PSUM tile; C(prev)'s ~1.7us of independent PE
            # work sits between).  Fillers hold the remaining projections;
            # out-proj stripes of tile t start 2 steps after its last norm.
            kproj()
            qproj(0)

            fillers = {
                1: lambda: qproj(1),
                2: lambda: qproj(2),
                4: lambda: qproj(3),
            }
            ostripes = {
                4: (0, (0, 1)),
                5: (0, (2, 3)),
                7: (1, (0, 1)),
                8: (1, (2, 3)),
                10: (2, (0, 1)),
                11: (2, (2, 3)),
            }

            ctxts = {}
            prev = None
            for k in range(NSTEP):
                t, p = divmod(k, MC)
                if p == 0:
                    ctxts[t] = ctxp.tile(
                        [P, MC, SQT], bf16, tag="ctxt", name=f"ctxt{t}"
                    )
                ets = etsp.tile([P, 2, SC, SQT], bf16, tag="ets", name=f"e{t}_{p}")
                s_round(t, p, 0, ets)
                if prev is None:
                    vproj(range(SC))  # fills the exp-r0 shadow at k=0
                else:
                    pt, pp, pets = prev
                    pcps = ctx_mm(pt, pp, pets)
                s_round(t, p, 1, ets)
                if prev is not None:
                    norm(pt, pp, pcps, ctxts[pt])
                if k in fillers:
                    fillers[k]()
                if k in ostripes:
                    ot_t, mqs = ostripes[k]
                    for mq in mqs:
                        oproj_stripe(ot_t, ctxts[ot_t], mq)
                prev = (t, p, ets)
            # tail
            pt, pp, pets = prev
            pcps = ctx_mm(pt, pp, pets)
            norm(pt, pp, pcps, ctxts[pt])
            for mq in range(NSQT):
                oproj_stripe(3, ctxts[3], mq)
            if DEBUG:
                nc.sync.dma_start(dbg_qt[:, :, :], qt)
                nc.sync.dma_start(dbg_ktp[:, :, :], ktp)
                nc.sync.dma_start(dbg_vb[:, :, :, :], vb)

    nc.compile()
    return nc


def _get_nc():
    if "nc" not in _CACHE:
        _CACHE["nc"] = _build_bass()
    return _CACHE["nc"]


def kernel(query, key, value, mask, uniform_set, Wq, bq, Wk, bk, Wv, bv, Wo, bo):
    import ml_dtypes
    from concourse import bass_utils

    bft = ml_dtypes.bfloat16

    query = np.asarray(query, dtype=np.float32)
    key = np.asarray(key, dtype=np.float32)
    value = np.asarray(value, dtype=np.float32)
    mask = np.asarray(mask, dtype=np.float32)
    us = np.asarray(uniform_set).astype(bool)
    Wq = np.asarray(Wq, dtype=np.float32)
    Wk = np.asarray(Wk, dtype=np.float32)
    Wv = np.asarray(Wv, dtype=np.float32)
    Wo = np.asarray(Wo, dtype=np.float32)
    bq = np.asarray(bq, dtype=np.float32)
    bk = np.asarray(bk, dtype=np.float32)
    bv = np.asarray(bv, dtype=np.float32)
    bo = np.asarray(bo, dtype=np.float32)
    assert np.all(bq == 0.0), "kernel assumes bq == 0 (reference generates zeros)"

    nc = _get_nc()

    scale = 1.0 / float(HD) ** 0.5
    wqt_g = [np.ascontiguousarray((Wq.T[:, g * DG : (g + 1) * DG] * scale)).astype(bft) for g in range(HG)]
    wkt_g = [np.ascontiguousarray(Wk.T[:, g * DG : (g + 1) * DG]).astype(bft) for g in range(HG)]
    wvt_g = [np.ascontiguousarray(Wv.T[:, g * DG : (g + 1) * DG]).astype(bft) for g in range(HG)]
    wot_g = [np.ascontiguousarray(Wo.T[g * DG : (g + 1) * DG, :]).astype(bft) for g in range(HG)]

    in_maps = []
    for b in range(B):
        keep = us & (mask[b, 0, 0] >= 0)
        idx = np.nonzero(keep)[0]
        n = len(idx)
        assert 0 < n <= NK, f"selected key count {n} unsupported"
        kselt = np.zeros((D, NK), bft)
        kselt[:, :n] = key[b][idx].T.astype(bft)
        vselt = np.zeros((D, NK), bft)
        vselt[:, :n] = value[b][idx].T.astype(bft)
        kflag = np.zeros((NK,), bft)
        kflag[:n] = 1.0
        xt = np.ascontiguousarray(query[b].T).astype(bft)
        for g in range(HG):
            in_maps.append(
                {
                    "xt": xt,
                    "kselt": kselt,
                    "vselt": vselt,
                    "wqt": wqt_g[g],
                    "wkt": wkt_g[g],
                    "wvt": wvt_g[g],
                    "wot": wot_g[g],
                    "kflag": kflag,
                }
            )

    res = bass_utils.run_bass_kernel_spmd(nc, in_maps, core_ids=list(range(B * HG)))
    outs = [m["out"] for m in res.results]

    corr = (bo + Wo @ bv).astype(np.float32)
    out = np.empty((B, S, D), np.float32)
    for b in range(B):
        out[b] = outs[HG * b].astype(np.float32) + outs[HG * b + 1].astype(np.float32) + corr
    return out
